# revision 9
# baseline (speedup 1.0000x reference)
"""Trainium2 Bass kernel for nn_ConcatLayer: (N, 9) -> (N, 3).

Pure data-parallel: the batch dim is sharded across 8 NeuronCores; each core
runs an identical elementwise Bass/Tile program over its shard.

Algorithm (equal to the reference on continuous inputs; within-segment exact
ties — measure-zero for randn data — may resolve differently):
  per row v(9,), segments s with components a/b/c:
    mx1 = max(b, c); mxF = max(a, mx1)
    m_s = (a > mx1) - (c == mxF)                     in {-1,0,1}
    calc = m_n^2 * (m_u + m_n + m_d); sgn = clip(calc, -1, 1)
    SEL_s = c_s if calc >= 2 else mxF_s   (== column-col value for alive segs)
    z_s  = (m_s == sgn); cmp_s = z_s * SEL_s
    row  = first argmax(cmp_u, cmp_n, cmp_d)
    out  = v[row] * z_row

Layout/scheduling strategy (from trace analysis):
  - The Vector engine runs ~4.5x slower on access patterns whose inner dim is
    strided, so the input is deinterleaved once into per-component planes
    [P, c, s, f] and every Vector op then streams fully dense APs.
  - The deinterleave (plus the output-default copy) runs on the Scalar engine,
    which is contention-free with Vector, software-pipelined one tile ahead so
    Vector never waits on it.
  - GpSimd is left idle: it shares the SBUF port with the Vector engine and
    measurably slows it (~2.8x on dense fp32 TT) when active.
"""

import numpy as np

import concourse.bass as bass
import concourse.mybir as mybir
from concourse.alu_op_type import AluOpType as A
from concourse.tile import TileContext
from concourse.bass_utils import run_bass_kernel_spmd

P = 128
N_CORES = 8
FP32 = mybir.dt.float32
BF16 = mybir.dt.bfloat16
U8 = mybir.dt.uint8
ACT = mybir.ActivationFunctionType


def build_kernel(rows_per_core: int, f: int) -> bass.Bass:
    """Build the per-core Bass program. rows_per_core must equal 128*f*ntiles."""
    assert rows_per_core % (P * f) == 0
    ntiles = rows_per_core // (P * f)

    nc = bass.Bass()
    x = nc.declare_dram_parameter("x", [rows_per_core, 9], FP32, isOutput=False)
    y = nc.declare_dram_parameter("y", [rows_per_core, 3], FP32, isOutput=True)

    with TileContext(nc) as tc:
        with (
            tc.tile_pool(name="io", bufs=2) as io,
            tc.tile_pool(name="wk", bufs=2) as wk,
        ):
            xts = {}
            xPs = {}

            def load_and_deint(t, chunks=1, dve_assist=False):
                """DMA tile t in and deinterleave it on the Scalar engine.

                chunks > 1 splits the DMA and deinterleave into row-chunks so
                deinterleaving starts before the full tile has landed; with
                dve_assist the (otherwise idle) Vector engine takes two of the
                three planes.  Both are used for tile 0 to shorten the ramp.
                """
                r0 = t * P * f
                r1 = (t + 1) * P * f
                fq = f // chunks
                xt = io.tile([P, f * 9], FP32, tag="xt")
                xP = wk.tile([P, 3, 3, f], FP32, tag="xP")
                xq = x[r0:r1, :].rearrange("(p q fq) c -> p q (fq c)", p=P, q=chunks)
                for q in range(chunks):
                    xtq = xt[:, q * fq * 9:(q + 1) * fq * 9]
                    nc.sync.dma_start(out=xtq, in_=xq[:, q, :])
                    Rcs = xtq.rearrange("p (fq s c) -> p c s fq", s=3, c=3)
                    for cidx in (1, 2, 0):
                        eng = (
                            nc.vector.tensor_copy
                            if dve_assist and cidx != 1
                            else nc.scalar.copy
                        )
                        kw = (
                            {"out": xP[:, cidx, :, q * fq:(q + 1) * fq],
                             "in_": Rcs[:, cidx]}
                        )
                        if eng is nc.scalar.copy:
                            nc.scalar.copy(**kw)
                        else:
                            nc.vector.tensor_copy(out=kw["out"], in_=kw["in_"])
                xts[t] = xt
                xPs[t] = xP

            load_and_deint(0, chunks=2, dve_assist=True)
            for t in range(ntiles):
                r0 = t * P * f
                r1 = (t + 1) * P * f
                xt, xP = xts.pop(t), xPs.pop(t)
                aP, bP, cP = xP[:, 0], xP[:, 1], xP[:, 2]
                Rsc = xt[:].rearrange("p (f s c) -> p s f c", s=3, c=3)
                U, Nv, D = Rsc[:, 0], Rsc[:, 1], Rsc[:, 2]

                # Scalar engine: default-output copy for THIS tile, then
                # prefetch-deinterleave for the NEXT tile.
                ot = io.tile([P, f * 3], FP32, tag="ot")
                O3 = ot[:].rearrange("p (f c) -> p f c", c=3)
                nc.scalar.copy(out=O3, in_=D)
                if t + 1 < ntiles:
                    load_and_deint(t + 1)

                # --- segment maxes and max-index m (planes, all dense) ---
                mx1 = wk.tile([P, 3, f], FP32, tag="mx1")
                mxF = wk.tile([P, 3, f], FP32, tag="mxF")
                Pt = wk.tile([P, 3, f], BF16, tag="Pt")
                Qt = wk.tile([P, 3, f], BF16, tag="Qt")
                Mt = wk.tile([P, 3, f], BF16, tag="Mt")
                nc.vector.tensor_tensor(out=mx1[:], in0=bP, in1=cP, op=A.max)
                nc.vector.tensor_tensor(out=mxF[:], in0=aP, in1=mx1[:], op=A.max)
                nc.vector.tensor_tensor(out=Pt[:], in0=aP, in1=mx1[:], op=A.is_gt)
                nc.vector.tensor_tensor(out=Qt[:], in0=cP, in1=mxF[:], op=A.is_equal)
                # bf16 dense TT gets the 2x perf mode (the STT variant is 1x)
                nc.vector.tensor_tensor(
                    out=Mt[:], in0=Pt[:], in1=Qt[:], op=A.subtract
                )
                m_u, m_n, m_d = Mt[:, 0, :], Mt[:, 1, :], Mt[:, 2, :]

                # --- row-level scalars (all dense smalls, bf16-exact ints) ---
                sm = wk.tile([P, 8, f], BF16, tag="sm")
                (t_s, S_s, an_s, calc_s, sgn_s, nbu_s, gnd_s, _sp) = (
                    sm[:, i, :] for i in range(8)
                )
                msk = wk.tile([P, 3, f], U8, tag="msk")
                cge2_s, bu_s, bn_s = (msk[:, i, :] for i in range(3))

                nc.vector.tensor_tensor(out=t_s, in0=m_u, in1=m_d, op=A.add)
                nc.vector.tensor_tensor(out=S_s, in0=t_s, in1=m_n, op=A.add)
                nc.vector.tensor_tensor(out=an_s, in0=m_n, in1=m_n, op=A.mult)
                nc.vector.tensor_tensor(out=calc_s, in0=an_s, in1=S_s, op=A.mult)
                nc.vector.tensor_scalar(
                    out=sgn_s, in0=calc_s, scalar1=-1.0, scalar2=1.0,
                    op0=A.max, op1=A.min,
                )
                nc.vector.tensor_scalar(
                    out=cge2_s, in0=calc_s, scalar1=2.0, scalar2=None, op0=A.is_ge
                )

                # --- SEL: mxF overwritten with the c-plane where calc >= 2 ---
                nc.vector.copy_predicated(
                    out=mxF[:],
                    mask=cge2_s.unsqueeze(1).to_broadcast([P, 3, f]),
                    data=cP,
                )

                # --- alive gates and gated comparands ---
                zt = wk.tile([P, 3, f], BF16, tag="zt")
                for s in range(3):
                    nc.vector.tensor_tensor(
                        out=zt[:, s, :], in0=Mt[:, s, :], in1=sgn_s, op=A.is_equal
                    )
                CMP = wk.tile([P, 3, f], FP32, tag="CMP")
                nc.vector.tensor_tensor(out=CMP[:], in0=zt[:], in1=mxF[:], op=A.mult)
                cmp_u, cmp_n, cmp_d = CMP[:, 0, :], CMP[:, 1, :], CMP[:, 2, :]

                # --- first-argmax row masks ---
                mxnd = wk.tile([P, 1, f], FP32, tag="mxnd")
                nc.vector.tensor_tensor(
                    out=mxnd[:, 0, :], in0=cmp_n, in1=cmp_d, op=A.max
                )
                nc.vector.tensor_tensor(
                    out=bu_s, in0=cmp_u, in1=mxnd[:, 0, :], op=A.is_ge
                )
                nc.vector.tensor_tensor(out=gnd_s, in0=cmp_n, in1=cmp_d, op=A.is_ge)
                nc.vector.tensor_scalar(
                    out=nbu_s, in0=bu_s, scalar1=-1.0, scalar2=1.0,
                    op0=A.mult, op1=A.add,
                )
                nc.vector.tensor_tensor(out=bn_s, in0=nbu_s, in1=gnd_s, op=A.mult)

                # --- winner z gate: predicated in place into z_d ---
                nc.vector.copy_predicated(out=zt[:, 2, :], mask=bn_s, data=zt[:, 1, :])
                nc.vector.copy_predicated(out=zt[:, 2, :], mask=bu_s, data=zt[:, 0, :])

                # --- output: winner segment (runs-of-3 APs) * zw, in place ---
                nc.vector.copy_predicated(
                    out=O3, mask=bn_s.broadcast_to([P, f, 3]), data=Nv
                )
                nc.vector.copy_predicated(
                    out=O3, mask=bu_s.broadcast_to([P, f, 3]), data=U
                )
                nc.vector.tensor_tensor(
                    out=O3, in0=O3, in1=zt[:, 2, :].broadcast_to([P, f, 3]), op=A.mult
                )

                nc.scalar.dma_start(
                    out=y[r0:r1, :].rearrange("(p f) c -> p (f c)", p=P),
                    in_=ot[:],
                )

    return nc


def legalize_multi_waits(nc: bass.Bass) -> None:
    """Split multi-wait sync_info into standalone EventSemaphore instructions.

    The walrus build in this environment encodes at most ONE sync-wait per
    instruction ("Too many sync wait commands" in codegen otherwise), while
    Tile emits one wait per depended-on semaphore.  Hoist all but the last
    wait onto dedicated same-engine wait instructions placed immediately
    before, which preserves per-engine program order and thus semantics.
    """
    n = 0
    for fn in nc.m.functions:
        for bb in fn.blocks:
            new_insts = []
            for inst in bb.instructions:
                si = inst.sync_info
                if si is not None and si.on_wait and len(si.on_wait) > 1:
                    waits = list(si.on_wait)
                    for w in waits[:-1]:
                        n += 1
                        new_insts.append(
                            mybir.InstEventSemaphore(
                                name=f"WSPLIT-{n}",
                                engine=inst.engine,
                                ins=[],
                                outs=[],
                                sync_info=mybir.SyncInfo(
                                    on_wait=[w], on_update=[]
                                ),
                            )
                        )
                    inst.sync_info = mybir.SyncInfo(
                        on_wait=[waits[-1]], on_update=list(si.on_update)
                    )
                new_insts.append(inst)
            bb.instructions = new_insts


_CACHED = {}


def _get_kernel(rows_per_core: int, f: int) -> bass.Bass:
    key = (rows_per_core, f)
    if key not in _CACHED:
        nc = build_kernel(rows_per_core, f)
        nc.finalize()
        legalize_multi_waits(nc)
        _CACHED[key] = nc
    return _CACHED[key]


def kernel(x: np.ndarray) -> np.ndarray:
    x = np.ascontiguousarray(np.asarray(x), dtype=np.float32)
    n = x.shape[0]
    assert n % N_CORES == 0
    rpc = n // N_CORES
    f = 512
    nc = _get_kernel(rpc, f)
    shards = [x[i * rpc:(i + 1) * rpc] for i in range(N_CORES)]
    in_maps = [{"x": s} for s in shards]
    res = run_bass_kernel_spmd(nc, in_maps, list(range(N_CORES))).results
    return np.concatenate([r["y"] for r in res], axis=0)


def run_traced(x: np.ndarray, f: int = 512):
    """Test-only: run with NTFF profiling, return BassKernelResults."""
    x = np.ascontiguousarray(np.asarray(x), dtype=np.float32)
    n = x.shape[0]
    rpc = n // N_CORES
    nc = _get_kernel(rpc, f)
    shards = [x[i * rpc:(i + 1) * rpc] for i in range(N_CORES)]
    in_maps = [{"x": s} for s in shards]
    return run_bass_kernel_spmd(
        nc, in_maps, list(range(N_CORES)), trace=True, trace_cores=[0]
    )


# revision 10
# speedup vs baseline: 1.2518x; 1.2518x over previous
"""Trainium2 Bass kernel for nn_ConcatLayer: (N, 9) -> (N, 3).

Pure data-parallel: the batch dim is sharded across 8 NeuronCores; each core
runs an identical elementwise Bass/Tile program over its shard.

Algorithm (equal to the reference on continuous inputs; within-segment exact
ties — measure-zero for randn data — may resolve differently):
  per row v(9,), segments s with components a/b/c:
    mx1 = max(b, c); mxF = max(a, mx1)
    m_s = (a > mx1) - (c == mxF)                     in {-1,0,1}
    calc = m_n^2 * (m_u + m_n + m_d); sgn = clip(calc, -1, 1)
    SEL_s = c_s if calc >= 2 else mxF_s   (== column-col value for alive segs)
    z_s  = (m_s == sgn); cmp_s = z_s * SEL_s
    row  = first argmax(cmp_u, cmp_n, cmp_d)
    out  = v[row] * z_row

Layout/scheduling strategy (from trace analysis):
  - The Vector engine runs ~4.5x slower on access patterns whose inner dim is
    strided, so the input is deinterleaved once into per-component planes
    [P, c, s, f] and every Vector op then streams fully dense APs.
  - The deinterleave (plus the output-default copy) runs on the Scalar engine,
    which is contention-free with Vector, software-pipelined one tile ahead so
    Vector never waits on it.
  - GpSimd is left idle: it shares the SBUF port with the Vector engine and
    measurably slows it (~2.8x on dense fp32 TT) when active.
"""

import numpy as np

import concourse.bass as bass
import concourse.mybir as mybir
from concourse.alu_op_type import AluOpType as A
from concourse.tile import TileContext
from concourse.bass_utils import run_bass_kernel_spmd

P = 128
N_CORES = 8
FP32 = mybir.dt.float32
BF16 = mybir.dt.bfloat16
U8 = mybir.dt.uint8
ACT = mybir.ActivationFunctionType


def build_kernel(rows_per_core: int, f: int) -> bass.Bass:
    """Build the per-core Bass program. rows_per_core must equal 128*f*ntiles."""
    assert rows_per_core % (P * f) == 0
    ntiles = rows_per_core // (P * f)

    nc = bass.Bass()
    x = nc.declare_dram_parameter("x", [rows_per_core, 9], FP32, isOutput=False)
    y = nc.declare_dram_parameter("y", [rows_per_core, 3], FP32, isOutput=True)

    with TileContext(nc) as tc:
        with (
            tc.tile_pool(name="io", bufs=2) as io,
            tc.tile_pool(name="wk", bufs=2) as wk,
        ):
            xts = {}
            xPs = {}

            def load_and_deint(t, chunks=1, dve_assist=False):
                """DMA tile t in and deinterleave it on the Scalar engine.

                chunks > 1 splits the DMA and deinterleave into row-chunks so
                deinterleaving starts before the full tile has landed; with
                dve_assist the (otherwise idle) Vector engine takes two of the
                three planes.  Both are used for tile 0 to shorten the ramp.
                """
                r0 = t * P * f
                r1 = (t + 1) * P * f
                fq = f // chunks
                xt = io.tile([P, f * 9], FP32, tag="xt")
                xP = wk.tile([P, 3, 3, f], FP32, tag="xP")
                xq = x[r0:r1, :].rearrange("(p q fq) c -> p q (fq c)", p=P, q=chunks)
                for q in range(chunks):
                    xtq = xt[:, q * fq * 9:(q + 1) * fq * 9]
                    nc.sync.dma_start(out=xtq, in_=xq[:, q, :])
                    Rcs = xtq.rearrange("p (fq s c) -> p c s fq", s=3, c=3)
                    for cidx in (1, 2, 0):
                        on_dve = dve_assist and cidx != 1
                        dst = xP[:, cidx, :, q * fq:(q + 1) * fq]
                        if on_dve:
                            nc.vector.tensor_copy(out=dst, in_=Rcs[:, cidx])
                        else:
                            nc.scalar.copy(out=dst, in_=Rcs[:, cidx])
                xts[t] = xt
                xPs[t] = xP

            load_and_deint(0, chunks=2, dve_assist=True)
            for t in range(ntiles):
                r0 = t * P * f
                r1 = (t + 1) * P * f
                xt, xP = xts.pop(t), xPs.pop(t)
                aP, bP, cP = xP[:, 0], xP[:, 1], xP[:, 2]
                Rsc = xt[:].rearrange("p (f s c) -> p s f c", s=3, c=3)
                U, Nv, D = Rsc[:, 0], Rsc[:, 1], Rsc[:, 2]

                # Scalar engine: default-output copy for THIS tile, then
                # prefetch-deinterleave for the NEXT tile.
                ot = io.tile([P, f * 3], FP32, tag="ot")
                O3 = ot[:].rearrange("p (f c) -> p f c", c=3)
                nc.scalar.copy(out=O3, in_=D)
                if t + 1 < ntiles:
                    load_and_deint(t + 1)

                # --- segment maxes and max-index m (planes, all dense) ---
                mx1 = wk.tile([P, 3, f], FP32, tag="mx1")
                mxF = wk.tile([P, 3, f], FP32, tag="mxF")
                Pt = wk.tile([P, 3, f], BF16, tag="Pt")
                Qt = wk.tile([P, 3, f], BF16, tag="Qt")
                Mt = wk.tile([P, 3, f], BF16, tag="Mt")
                nc.vector.tensor_tensor(out=mx1[:], in0=bP, in1=cP, op=A.max)
                nc.vector.tensor_tensor(out=mxF[:], in0=aP, in1=mx1[:], op=A.max)
                nc.vector.tensor_tensor(out=Pt[:], in0=aP, in1=mx1[:], op=A.is_gt)
                nc.vector.tensor_tensor(out=Qt[:], in0=cP, in1=mxF[:], op=A.is_equal)
                # bf16 dense TT gets the 2x perf mode (the STT variant is 1x)
                nc.vector.tensor_tensor(
                    out=Mt[:], in0=Pt[:], in1=Qt[:], op=A.subtract
                )
                m_u, m_n, m_d = Mt[:, 0, :], Mt[:, 1, :], Mt[:, 2, :]

                # --- row-level scalars (all dense smalls, bf16-exact ints) ---
                sm = wk.tile([P, 8, f], BF16, tag="sm")
                (t_s, S_s, an_s, calc_s, sgn_s, nbu_s, gnd_s, _sp) = (
                    sm[:, i, :] for i in range(8)
                )
                msk = wk.tile([P, 3, f], U8, tag="msk")
                cge2_s, bu_s, bn_s = (msk[:, i, :] for i in range(3))

                nc.vector.tensor_tensor(out=t_s, in0=m_u, in1=m_d, op=A.add)
                nc.vector.tensor_tensor(out=S_s, in0=t_s, in1=m_n, op=A.add)
                nc.vector.tensor_tensor(out=an_s, in0=m_n, in1=m_n, op=A.mult)
                nc.vector.tensor_tensor(out=calc_s, in0=an_s, in1=S_s, op=A.mult)
                nc.vector.tensor_scalar(
                    out=sgn_s, in0=calc_s, scalar1=-1.0, scalar2=1.0,
                    op0=A.max, op1=A.min,
                )
                nc.vector.tensor_scalar(
                    out=cge2_s, in0=calc_s, scalar1=2.0, scalar2=None, op0=A.is_ge
                )

                # --- SEL: mxF overwritten with the c-plane where calc >= 2 ---
                nc.vector.copy_predicated(
                    out=mxF[:],
                    mask=cge2_s.unsqueeze(1).to_broadcast([P, 3, f]),
                    data=cP,
                )

                # --- alive gates and gated comparands ---
                zt = wk.tile([P, 3, f], BF16, tag="zt")
                for s in range(3):
                    nc.vector.tensor_tensor(
                        out=zt[:, s, :], in0=Mt[:, s, :], in1=sgn_s, op=A.is_equal
                    )
                CMP = wk.tile([P, 3, f], FP32, tag="CMP")
                nc.vector.tensor_tensor(out=CMP[:], in0=zt[:], in1=mxF[:], op=A.mult)
                cmp_u, cmp_n, cmp_d = CMP[:, 0, :], CMP[:, 1, :], CMP[:, 2, :]

                # --- first-argmax row masks ---
                mxnd = wk.tile([P, 1, f], FP32, tag="mxnd")
                nc.vector.tensor_tensor(
                    out=mxnd[:, 0, :], in0=cmp_n, in1=cmp_d, op=A.max
                )
                nc.vector.tensor_tensor(
                    out=bu_s, in0=cmp_u, in1=mxnd[:, 0, :], op=A.is_ge
                )
                nc.vector.tensor_tensor(out=gnd_s, in0=cmp_n, in1=cmp_d, op=A.is_ge)
                nc.vector.tensor_scalar(
                    out=nbu_s, in0=bu_s, scalar1=-1.0, scalar2=1.0,
                    op0=A.mult, op1=A.add,
                )
                nc.vector.tensor_tensor(out=bn_s, in0=nbu_s, in1=gnd_s, op=A.mult)

                # --- winner z gate: predicated in place into z_d ---
                nc.vector.copy_predicated(out=zt[:, 2, :], mask=bn_s, data=zt[:, 1, :])
                nc.vector.copy_predicated(out=zt[:, 2, :], mask=bu_s, data=zt[:, 0, :])

                # --- output: winner segment (runs-of-3 APs) * zw, in place ---
                nc.vector.copy_predicated(
                    out=O3, mask=bn_s.broadcast_to([P, f, 3]), data=Nv
                )
                nc.vector.copy_predicated(
                    out=O3, mask=bu_s.broadcast_to([P, f, 3]), data=U
                )
                nc.vector.tensor_tensor(
                    out=O3, in0=O3, in1=zt[:, 2, :].broadcast_to([P, f, 3]), op=A.mult
                )

                nc.scalar.dma_start(
                    out=y[r0:r1, :].rearrange("(p f) c -> p (f c)", p=P),
                    in_=ot[:],
                )

    return nc


def legalize_multi_waits(nc: bass.Bass) -> None:
    """Split multi-wait sync_info into standalone EventSemaphore instructions.

    The walrus build in this environment encodes at most ONE sync-wait per
    instruction ("Too many sync wait commands" in codegen otherwise), while
    Tile emits one wait per depended-on semaphore.  Hoist all but the last
    wait onto dedicated same-engine wait instructions placed immediately
    before, which preserves per-engine program order and thus semantics.
    """
    n = 0
    for fn in nc.m.functions:
        for bb in fn.blocks:
            new_insts = []
            for inst in bb.instructions:
                si = inst.sync_info
                if si is not None and si.on_wait and len(si.on_wait) > 1:
                    waits = list(si.on_wait)
                    for w in waits[:-1]:
                        n += 1
                        new_insts.append(
                            mybir.InstEventSemaphore(
                                name=f"WSPLIT-{n}",
                                engine=inst.engine,
                                ins=[],
                                outs=[],
                                sync_info=mybir.SyncInfo(
                                    on_wait=[w], on_update=[]
                                ),
                            )
                        )
                    inst.sync_info = mybir.SyncInfo(
                        on_wait=[waits[-1]], on_update=list(si.on_update)
                    )
                new_insts.append(inst)
            bb.instructions = new_insts


_CACHED = {}


def _get_kernel(rows_per_core: int, f: int) -> bass.Bass:
    key = (rows_per_core, f)
    if key not in _CACHED:
        nc = build_kernel(rows_per_core, f)
        nc.finalize()
        legalize_multi_waits(nc)
        _CACHED[key] = nc
    return _CACHED[key]


def kernel(x: np.ndarray) -> np.ndarray:
    x = np.ascontiguousarray(np.asarray(x), dtype=np.float32)
    n = x.shape[0]
    assert n % N_CORES == 0
    rpc = n // N_CORES
    f = 512
    nc = _get_kernel(rpc, f)
    shards = [x[i * rpc:(i + 1) * rpc] for i in range(N_CORES)]
    in_maps = [{"x": s} for s in shards]
    res = run_bass_kernel_spmd(nc, in_maps, list(range(N_CORES))).results
    return np.concatenate([r["y"] for r in res], axis=0)


def run_traced(x: np.ndarray, f: int = 512):
    """Test-only: run with NTFF profiling, return BassKernelResults."""
    x = np.ascontiguousarray(np.asarray(x), dtype=np.float32)
    n = x.shape[0]
    rpc = n // N_CORES
    nc = _get_kernel(rpc, f)
    shards = [x[i * rpc:(i + 1) * rpc] for i in range(N_CORES)]
    in_maps = [{"x": s} for s in shards]
    return run_bass_kernel_spmd(
        nc, in_maps, list(range(N_CORES)), trace=True, trace_cores=[0]
    )


# revision 11
# speedup vs baseline: 1.3421x; 1.0721x over previous
"""Trainium2 Bass kernel for nn_ConcatLayer: (N, 9) -> (N, 3).

Pure data-parallel: the batch dim is sharded across 8 NeuronCores; each core
runs an identical elementwise Bass/Tile program over its shard.

Algorithm (equal to the reference on continuous inputs; within-segment exact
ties — measure-zero for randn data — may resolve differently):
  per row v(9,), segments s with components a/b/c:
    mx1 = max(b, c); mxF = max(a, mx1)
    m_s = (a > mx1) - (c == mxF)                     in {-1,0,1}
    calc = m_n^2 * (m_u + m_n + m_d); sgn = clip(calc, -1, 1)
    SEL_s = c_s if calc >= 2 else mxF_s   (== column-col value for alive segs)
    z_s  = (m_s == sgn); cmp_s = z_s * SEL_s
    row  = first argmax(cmp_u, cmp_n, cmp_d)
    out  = v[row] * z_row

Layout/scheduling strategy (from trace analysis):
  - The Vector engine runs ~4.5x slower on access patterns whose inner dim is
    strided, so the input is deinterleaved once into per-component planes
    [P, c, s, f] and every Vector op then streams fully dense APs.
  - The deinterleave (plus the output-default copy) runs on the Scalar engine,
    which is contention-free with Vector, software-pipelined one tile ahead so
    Vector never waits on it.
  - GpSimd is left idle: it shares the SBUF port with the Vector engine and
    measurably slows it (~2.8x on dense fp32 TT) when active.
"""

import numpy as np

import concourse.bass as bass
import concourse.mybir as mybir
from concourse.alu_op_type import AluOpType as A
from concourse.tile import TileContext
from concourse.bass_utils import run_bass_kernel_spmd

P = 128
N_CORES = 8
FP32 = mybir.dt.float32
BF16 = mybir.dt.bfloat16
U8 = mybir.dt.uint8
ACT = mybir.ActivationFunctionType


def build_kernel(rows_per_core: int, f: int) -> bass.Bass:
    """Build the per-core Bass program. rows_per_core must equal 128*f*ntiles."""
    assert rows_per_core % (P * f) == 0
    ntiles = rows_per_core // (P * f)

    nc = bass.Bass()
    x = nc.declare_dram_parameter("x", [rows_per_core, 9], FP32, isOutput=False)
    y = nc.declare_dram_parameter("y", [rows_per_core, 3], FP32, isOutput=True)

    with TileContext(nc) as tc:
        with (
            tc.tile_pool(name="xin", bufs=3) as xin,
            tc.tile_pool(name="io", bufs=2) as io,
            tc.tile_pool(name="wk", bufs=2) as wk,
        ):
            xts = {}
            xPs = {}

            def dma_in(t, chunks=1):
                r0 = t * P * f
                r1 = (t + 1) * P * f
                fq = f // chunks
                xt = xin.tile([P, f * 9], FP32, tag="xt")
                xq = x[r0:r1, :].rearrange(
                    "(p q fq) c -> p q (fq c)", p=P, q=chunks
                )
                for q in range(chunks):
                    nc.sync.dma_start(
                        out=xt[:, q * fq * 9:(q + 1) * fq * 9], in_=xq[:, q, :]
                    )
                xts[t] = xt

            def deint(t, chunks=1, dve_assist=False):
                """Deinterleave tile t into planes on the Scalar engine.

                chunks > 1 splits the work into row-chunks; dve_assist lets
                the (otherwise idle) Vector engine take two of the three
                planes.  Both are used for tile 0 to shorten the ramp.
                """
                fq = f // chunks
                xt = xts[t]
                xP = wk.tile([P, 3, 3, f], FP32, tag="xP")
                for q in range(chunks):
                    xtq = xt[:, q * fq * 9:(q + 1) * fq * 9]
                    Rcs = xtq.rearrange("p (fq s c) -> p c s fq", s=3, c=3)
                    for cidx in (1, 2, 0):
                        on_dve = dve_assist and cidx != 1
                        dst = xP[:, cidx, :, q * fq:(q + 1) * fq]
                        if on_dve:
                            nc.vector.tensor_copy(out=dst, in_=Rcs[:, cidx])
                        else:
                            nc.scalar.copy(out=dst, in_=Rcs[:, cidx])
                xPs[t] = xP

            dma_in(0, chunks=2)
            if ntiles > 1:
                dma_in(1)
            deint(0, chunks=2, dve_assist=True)
            for t in range(ntiles):
                r0 = t * P * f
                r1 = (t + 1) * P * f
                xt, xP = xts.pop(t), xPs.pop(t)
                aP, bP, cP = xP[:, 0], xP[:, 1], xP[:, 2]
                Rsc = xt[:].rearrange("p (f s c) -> p s f c", s=3, c=3)
                U, Nv, D = Rsc[:, 0], Rsc[:, 1], Rsc[:, 2]

                # Scalar engine: default-output copy for THIS tile, then
                # prefetch-deinterleave for the NEXT tile.
                ot = io.tile([P, f * 3], FP32, tag="ot")
                O3 = ot[:].rearrange("p (f c) -> p f c", c=3)
                nc.scalar.copy(out=O3, in_=D)
                if t + 1 < ntiles:
                    deint(t + 1)
                if t + 2 < ntiles:
                    dma_in(t + 2)

                # --- segment maxes and max-index m (planes, all dense) ---
                mx1 = wk.tile([P, 3, f], FP32, tag="mx1")
                mxF = wk.tile([P, 3, f], FP32, tag="mxF")
                Pt = wk.tile([P, 3, f], BF16, tag="Pt")
                Qt = wk.tile([P, 3, f], BF16, tag="Qt")
                Mt = wk.tile([P, 3, f], BF16, tag="Mt")
                nc.vector.tensor_tensor(out=mx1[:], in0=bP, in1=cP, op=A.max)
                nc.vector.tensor_tensor(out=mxF[:], in0=aP, in1=mx1[:], op=A.max)
                nc.vector.tensor_tensor(out=Pt[:], in0=aP, in1=mx1[:], op=A.is_gt)
                nc.vector.tensor_tensor(out=Qt[:], in0=cP, in1=mxF[:], op=A.is_equal)
                # bf16 dense TT gets the 2x perf mode (the STT variant is 1x)
                nc.vector.tensor_tensor(
                    out=Mt[:], in0=Pt[:], in1=Qt[:], op=A.subtract
                )
                m_u, m_n, m_d = Mt[:, 0, :], Mt[:, 1, :], Mt[:, 2, :]

                # --- row-level scalars (all dense smalls, bf16-exact ints) ---
                sm = wk.tile([P, 8, f], BF16, tag="sm")
                (t_s, S_s, an_s, calc_s, sgn_s, nbu_s, gnd_s, _sp) = (
                    sm[:, i, :] for i in range(8)
                )
                msk = wk.tile([P, 3, f], U8, tag="msk")
                cge2_s, bu_s, bn_s = (msk[:, i, :] for i in range(3))

                nc.vector.tensor_tensor(out=t_s, in0=m_u, in1=m_d, op=A.add)
                nc.vector.tensor_tensor(out=S_s, in0=t_s, in1=m_n, op=A.add)
                nc.vector.tensor_tensor(out=an_s, in0=m_n, in1=m_n, op=A.mult)
                nc.vector.tensor_tensor(out=calc_s, in0=an_s, in1=S_s, op=A.mult)
                nc.vector.tensor_scalar(
                    out=sgn_s, in0=calc_s, scalar1=-1.0, scalar2=1.0,
                    op0=A.max, op1=A.min,
                )
                nc.vector.tensor_scalar(
                    out=cge2_s, in0=calc_s, scalar1=2.0, scalar2=None, op0=A.is_ge
                )

                # --- SEL: mxF overwritten with the c-plane where calc >= 2 ---
                nc.vector.copy_predicated(
                    out=mxF[:],
                    mask=cge2_s.unsqueeze(1).to_broadcast([P, 3, f]),
                    data=cP,
                )

                # --- alive gates and gated comparands ---
                zt = wk.tile([P, 3, f], BF16, tag="zt")
                for s in range(3):
                    nc.vector.tensor_tensor(
                        out=zt[:, s, :], in0=Mt[:, s, :], in1=sgn_s, op=A.is_equal
                    )
                CMP = wk.tile([P, 3, f], FP32, tag="CMP")
                nc.vector.tensor_tensor(out=CMP[:], in0=zt[:], in1=mxF[:], op=A.mult)
                cmp_u, cmp_n, cmp_d = CMP[:, 0, :], CMP[:, 1, :], CMP[:, 2, :]

                # --- first-argmax row masks ---
                mxnd = wk.tile([P, 1, f], FP32, tag="mxnd")
                nc.vector.tensor_tensor(
                    out=mxnd[:, 0, :], in0=cmp_n, in1=cmp_d, op=A.max
                )
                nc.vector.tensor_tensor(
                    out=bu_s, in0=cmp_u, in1=mxnd[:, 0, :], op=A.is_ge
                )
                nc.vector.tensor_tensor(out=gnd_s, in0=cmp_n, in1=cmp_d, op=A.is_ge)
                nc.vector.tensor_scalar(
                    out=nbu_s, in0=bu_s, scalar1=-1.0, scalar2=1.0,
                    op0=A.mult, op1=A.add,
                )
                nc.vector.tensor_tensor(out=bn_s, in0=nbu_s, in1=gnd_s, op=A.mult)

                # --- winner z gate: predicated in place into z_d ---
                nc.vector.copy_predicated(out=zt[:, 2, :], mask=bn_s, data=zt[:, 1, :])
                nc.vector.copy_predicated(out=zt[:, 2, :], mask=bu_s, data=zt[:, 0, :])

                # --- output: winner segment (runs-of-3 APs) * zw, in place ---
                nc.vector.copy_predicated(
                    out=O3, mask=bn_s.broadcast_to([P, f, 3]), data=Nv
                )
                nc.vector.copy_predicated(
                    out=O3, mask=bu_s.broadcast_to([P, f, 3]), data=U
                )
                nc.vector.tensor_tensor(
                    out=O3, in0=O3, in1=zt[:, 2, :].broadcast_to([P, f, 3]), op=A.mult
                )

                nc.scalar.dma_start(
                    out=y[r0:r1, :].rearrange("(p f) c -> p (f c)", p=P),
                    in_=ot[:],
                )

    return nc


def legalize_multi_waits(nc: bass.Bass) -> None:
    """Split multi-wait sync_info into standalone EventSemaphore instructions.

    The walrus build in this environment encodes at most ONE sync-wait per
    instruction ("Too many sync wait commands" in codegen otherwise), while
    Tile emits one wait per depended-on semaphore.  Hoist all but the last
    wait onto dedicated same-engine wait instructions placed immediately
    before, which preserves per-engine program order and thus semantics.
    """
    n = 0
    for fn in nc.m.functions:
        for bb in fn.blocks:
            new_insts = []
            for inst in bb.instructions:
                si = inst.sync_info
                if si is not None and si.on_wait and len(si.on_wait) > 1:
                    waits = list(si.on_wait)
                    for w in waits[:-1]:
                        n += 1
                        new_insts.append(
                            mybir.InstEventSemaphore(
                                name=f"WSPLIT-{n}",
                                engine=inst.engine,
                                ins=[],
                                outs=[],
                                sync_info=mybir.SyncInfo(
                                    on_wait=[w], on_update=[]
                                ),
                            )
                        )
                    inst.sync_info = mybir.SyncInfo(
                        on_wait=[waits[-1]], on_update=list(si.on_update)
                    )
                new_insts.append(inst)
            bb.instructions = new_insts


_CACHED = {}


def _get_kernel(rows_per_core: int, f: int) -> bass.Bass:
    key = (rows_per_core, f)
    if key not in _CACHED:
        nc = build_kernel(rows_per_core, f)
        nc.finalize()
        legalize_multi_waits(nc)
        _CACHED[key] = nc
    return _CACHED[key]


def kernel(x: np.ndarray) -> np.ndarray:
    x = np.ascontiguousarray(np.asarray(x), dtype=np.float32)
    n = x.shape[0]
    assert n % N_CORES == 0
    rpc = n // N_CORES
    f = 512
    nc = _get_kernel(rpc, f)
    shards = [x[i * rpc:(i + 1) * rpc] for i in range(N_CORES)]
    in_maps = [{"x": s} for s in shards]
    res = run_bass_kernel_spmd(nc, in_maps, list(range(N_CORES))).results
    return np.concatenate([r["y"] for r in res], axis=0)


def run_traced(x: np.ndarray, f: int = 512):
    """Test-only: run with NTFF profiling, return BassKernelResults."""
    x = np.ascontiguousarray(np.asarray(x), dtype=np.float32)
    n = x.shape[0]
    rpc = n // N_CORES
    nc = _get_kernel(rpc, f)
    shards = [x[i * rpc:(i + 1) * rpc] for i in range(N_CORES)]
    in_maps = [{"x": s} for s in shards]
    return run_bass_kernel_spmd(
        nc, in_maps, list(range(N_CORES)), trace=True, trace_cores=[0]
    )
